# revision 31
# baseline (speedup 1.0000x reference)
"""Fused LoRA-MLP (SwiGLU) expert kernel for TRN2, 8-core expert-parallel.

Problem (per full batch): x:(8192,2048) shared-weight expert MLP
    gu  = x @ W_gu.T + 0.25 * (x @ A_gu.T) @ B_gu.T        (.,8192)
    h   = gu[:, 4096:] * silu(gu[:, :4096])                 (.,4096)
    out = h @ W_d.T  + 0.25 * (h @ A_d.T)  @ B_d.T          (.,2048)

Sharding: expert/data parallel — core c owns tokens [1024c, 1024(c+1)),
weights replicated per core. No collectives.

All tensors are pre-transposed/pre-tiled on the host so the device kernel
needs zero on-chip transposes; activations flow feature-major
(xT -> guT -> hT -> outT). Weights and activations are bf16 (PSUM
accumulation stays fp32); output is written bf16 and upcast on host.

Schedule (tuned on the TimelineSim cost model, 743us -> 721us):
- opening: slab/x pieces stream smallest-first so PE starts at ~3us and
  is fed chunk-by-chunk; f-pair 0's chains + lora-1 are emitted in
  expected-readiness order (PE queue is FIFO)
- B_gu tiles all packed in rows 0:64 so both gate and up lora tails
  read xa rows 0:64 directly (no SBUF->SBUF duplication on the
  critical path)
- sync HWDGE queue carries pure streaming loads only; dependency-
  waiting DMAs (output stores) go on scalar so they cannot head-block
  the weight stream
- the final fused mm2 block spreads over ps_a's 4 PSUM banks and its
  16 bd tiles arrive as one batched load
"""

import os
from contextlib import ExitStack

import numpy as np

import concourse.bass as bass
import concourse.bacc as bacc
import concourse.tile as tile
import concourse.mybir as mybir
from concourse.bass_utils import run_bass_kernel_spmd

F32 = mybir.dt.float32
BF16 = mybir.dt.bfloat16
AF = mybir.ActivationFunctionType

NCORES = 8
T = 1024          # tokens per core
H = 2048          # hidden
D = 4096          # expert dim
F = 2 * D         # gate+up features
R = 64            # lora rank
SCALING = 16 / 64

KT = H // 128     # 16 k-tiles (mm1 contraction)
FT = F // 128     # 64 f-tiles (mm1 outputs)
DT = D // 128     # 32 d-tiles (mm2 contraction)
JT = H // 128     # 16 j-tiles (mm2 outputs)
NB = 8            # mm2 d-blocks (4 d-tiles each)
TC = 512          # moving-dim chunk
NCH = T // TC     # 2 chunks
SLAB = KT * 128 + 128  # wgu slab cols: 16 k-tiles + packed bgu tile

# PE-warmup dummy-matmul counts, keyed by opening slot (tuned on the
# TimelineSim cost model so the PE never idles during the opening)
WARM = {}
OPEN_ORDER = None
OPEN_COMPUTE = None

_CACHE = {}


def _build(reps=1, loop_n=None):
    nc = bacc.Bacc("TRN2", target_bir_lowering=False, debug=False,
                   num_devices=NCORES)

    xT = nc.dram_tensor("xT", [128, KT * T], BF16, kind="ExternalInput")
    wgu = nc.dram_tensor("wgu", [FT, 128, SLAB], BF16, kind="ExternalInput")
    agu = nc.dram_tensor("agu", [128, KT * R], BF16, kind="ExternalInput")
    wd = nc.dram_tensor("wd", [NB, JT, 128, 4 * 128], BF16, kind="ExternalInput")
    ad = nc.dram_tensor("ad", [128, DT * R], BF16, kind="ExternalInput")
    bd = nc.dram_tensor("bd", [128, JT * 128], BF16, kind="ExternalInput")
    outT = nc.dram_tensor("outT", [JT, 128, T], BF16, kind="ExternalOutput")

    with tile.TileContext(nc) as tc, ExitStack() as ctx:
        const = ctx.enter_context(tc.tile_pool(name="const", bufs=1))
        xpool = ctx.enter_context(tc.tile_pool(name="xpool", bufs=1))
        wgu_pool = ctx.enter_context(tc.tile_pool(name="wgup", bufs=4))
        wd_pool = ctx.enter_context(tc.tile_pool(name="wdp", bufs=4))
        bd_pool = ctx.enter_context(tc.tile_pool(name="bdp", bufs=4))
        ht_pool = ctx.enter_context(tc.tile_pool(name="htp", bufs=8))
        oacc_pool = ctx.enter_context(tc.tile_pool(name="oaccp", bufs=JT))
        obf_pool = ctx.enter_context(tc.tile_pool(name="obfp", bufs=4))
        sil_pool = ctx.enter_context(tc.tile_pool(name="silp", bufs=2))
        sm_pool = ctx.enter_context(tc.tile_pool(name="smp", bufs=1))
        ps_a = ctx.enter_context(tc.tile_pool(name="psa", bufs=4, space="PSUM"))
        ps_b = ctx.enter_context(tc.tile_pool(name="psb", bufs=2, space="PSUM"))
        ps_c = ctx.enter_context(tc.tile_pool(name="psc", bufs=2, space="PSUM"))

        def emit_rep(rep):
            # ---- opening DMA order is the critical path: tiny first
            # pieces of slab0/x so PE starts at ~2.5us, then the rest of
            # x streams in growing chunks; everything streaming on sync
            XCH = [1, 1, 2, 2, 2, 2, 2, 2, 2]    # k-tiles per x chunk
            xqof = []
            o = 0
            for n in XCH:
                xqof.append(o)
                o += n
            k2q = {}
            for q, (n, of) in enumerate(zip(XCH, xqof)):
                for kk in range(n):
                    k2q[of + kk] = (q, kk)
            xch = [xpool.tile([128, n * T], BF16, name=f"xch_{q}")
                   for q, n in enumerate(XCH)]

            # slab0 split: k0-1 piece first, remainder (k2-15 + B) after
            S0A = 2 * 128
            s0a = wgu_pool.tile([128, S0A], BF16, tag="wgua")
            s0b = wgu_pool.tile([128, SLAB - S0A], BF16, tag="wgub")
            abuf = const.tile([128, KT * R], BF16)
            s32 = wgu_pool.tile([128, SLAB], BF16, tag="wgu")
            pre_slabs = {}
            for m in (1, 1 + DT):
                pre_slabs[m] = wgu_pool.tile([128, SLAB], BF16, tag="wgu",
                                             name=f"preslab_{rep}_{m}")

            def _xq(q):
                nc.sync.dma_start(
                    out=xch[q][:], in_=xT[:, xqof[q] * T:(xqof[q] + XCH[q]) * T])
            _dma = {
                "s0a": lambda: nc.sync.dma_start(out=s0a[:],
                                                 in_=wgu[0, :, 0:S0A]),
                "s0b": lambda: nc.sync.dma_start(out=s0b[:],
                                                 in_=wgu[0, :, S0A:SLAB]),
                "abuf": lambda: nc.sync.dma_start(out=abuf[:], in_=agu[:, :]),
                "s32": lambda: nc.sync.dma_start(out=s32[:], in_=wgu[DT]),
                "pre1": lambda: nc.sync.dma_start(out=pre_slabs[1][:],
                                                  in_=wgu[1]),
                "pre33": lambda: nc.sync.dma_start(out=pre_slabs[1 + DT][:],
                                                   in_=wgu[1 + DT]),
            }
            order = OPEN_ORDER or (
                ["s0a", "x0", "abuf", "x1", "x2", "s0b", "x3", "s32"]
                + [f"x{q}" for q in range(4, len(XCH) - 1)]
                + ["pre1", "pre33", f"x{len(XCH) - 1}"])
            for item in order:
                if item.startswith("x"):
                    _xq(int(item[1:]))
                else:
                    _dma[item]()
            adbuf = const.tile([128, DT * R], BF16)

            def wsl0(k):
                # slab-0 column window for k-tile k (split across s0a/s0b)
                if k < 2:
                    return s0a[:, k * 128:(k + 1) * 128]
                return s0b[:, (k - 2) * 128:(k - 1) * 128]

            def xsl(k, c):
                q, kk = k2q[k]
                return xch[q][:, kk * T + c * TC: kk * T + (c + 1) * TC]

            # ---- PE warmup: dead matmuls on a memset tile keep the PE
            # busy from ~t=0 so the p-state ramp has elapsed (and the
            # engine never idles) by the time real matmuls dispatch
            if any(WARM.values()):
                wt = const.tile([128, 512], BF16)
                nc.vector.memset(wt[:], 0.0)
                pdum = ps_c.tile([128, TC], F32, tag="psc",
                                 name=f"pdum_{rep}")

            def dummies(n):
                for _ in range(n):
                    nc.tensor.matmul(pdum[:], wt[:, 0:128], wt[:, 0:512],
                                     start=True, stop=True)

            # ---- opening compute, emitted in expected-readiness order
            # (PE queue is FIFO): i=0 chains + lora-1, chunk-major
            pp0 = {m: [ps_a.tile([128, TC], F32, tag="psa",
                                 name=f"ps1_0_{m}_{c}")
                       for c in range(NCH)] for m in (0, DT)}
            pxa = [ps_b.tile([R, TC], F32, tag="psb", name=f"pxa_{rep}_{c}")
                   for c in range(NCH)]
            xa_sb = sm_pool.tile([R, T], BF16, tag="xa")

            def open_mm(m, k):
                w = wsl0(k) if m == 0 else s32[:, k * 128:(k + 1) * 128]
                for c in range(NCH):
                    nc.tensor.matmul(
                        pp0[m][c][:], w,
                        xsl(k, c), start=(k == 0), stop=False)

            def lora_mm(k):
                for c in range(NCH):
                    nc.tensor.matmul(
                        pxa[c][:], abuf[:, k * R:(k + 1) * R], xsl(k, c),
                        start=(k == 0), stop=(k == KT - 1))

            if OPEN_COMPUTE is not None:
                seq = OPEN_COMPUTE
            else:
                seq = [("o0", 0), ("o0", 1), ("o0", 2), ("o0", 3)]
                seq += [("lora", k) for k in range(4)]
                seq += [("oDT", k) for k in range(4)]
                for q in range(3, len(XCH)):
                    ks = range(xqof[q], xqof[q] + XCH[q])
                    seq += [("lora", k) for k in ks]
                    seq += [("o0", k) for k in ks]
                    seq += [("oDT", k) for k in ks]
            for op, k in seq:
                if op == "o0":
                    open_mm(0, k)
                elif op == "oDT":
                    open_mm(DT, k)
                elif op == "lora":
                    lora_mm(k)
                else:
                    dummies(k)

            for c in range(NCH):
                cs = slice(c * TC, (c + 1) * TC)
                nc.vector.tensor_copy(xa_sb[0:R, cs], pxa[c][:])

            # persistent xa2 accumulators (one full bank per chunk;
            # even d-tiles land in rows 0:64, odd in rows 64:128)
            pxa2 = [ps_c.tile([128, TC], F32, tag="psc", name=f"pxa2_{rep}_{c}")
                    for c in range(NCH)]

            ht_tiles = [None] * DT
            oacc = [None] * JT
            bdall = [None]

            def emit_xa2_mm(i):
                half = i % 2
                for c in range(NCH):
                    nc.tensor.matmul(
                        pxa2[c][half * R:(half + 1) * R, :],
                        adbuf[:, i * R:(i + 1) * R],
                        ht_tiles[i][:, c * TC:(c + 1) * TC],
                        start=(i < 2), stop=(i >= DT - 2),
                        tile_position=(0, half * R))

            def mm2_block(b, fuse_tail=False):
                for j in range(JT):
                    wdt = wd_pool.tile([128, 4 * 128], BF16, tag="wd")
                    (nc.sync if j % 2 else nc.scalar).dma_start(
                        out=wdt[:], in_=wd[b, j])
                    if fuse_tail:
                        bdt = bdall[0][:, j * 128:(j + 1) * 128]
                        obf = obf_pool.tile([128, T], BF16, tag="obf")
                    # both chunks under one weight load per d-tile; the
                    # fused final block runs after mm1 is done, so it can
                    # spread across ps_a's 4 banks instead of ps_b's 2
                    pool = ps_a if fuse_tail else ps_b
                    psl = [pool.tile([128, TC], F32, tag=pool.name,
                                     name=f"ps2_{b}_{j}_{c}")
                           for c in range(NCH)]
                    for dt_ in range(4):
                        d = b * 4 + dt_
                        for c in range(NCH):
                            nc.tensor.matmul(
                                psl[c][:], wdt[:, dt_ * 128:(dt_ + 1) * 128],
                                ht_tiles[d][:, c * TC:(c + 1) * TC],
                                start=(dt_ == 0), stop=(dt_ == 3 and not fuse_tail))
                    if fuse_tail:
                        # lora-2 tail folded into the last accumulation
                        for c in range(NCH):
                            nc.tensor.matmul(
                                psl[c][:], bdt[:],
                                xa2_sb[:, c * TC:(c + 1) * TC],
                                start=False, stop=True)
                    for c in range(NCH):
                        dst = oacc[j][:, c * TC:(c + 1) * TC]
                        if b == 0:
                            nc.vector.tensor_copy(dst, psl[c][:])
                        elif fuse_tail:
                            # per-chunk store so the final store is small
                            nc.vector.tensor_add(
                                obf[:, c * TC:(c + 1) * TC], dst, psl[c][:])
                            (nc.sync if j == JT - 1 else nc.scalar).dma_start(
                                out=outT[j][:, c * TC:(c + 1) * TC],
                                in_=obf[:, c * TC:(c + 1) * TC])
                        else:
                            nc.vector.tensor_add(dst, dst, psl[c][:])

            def close_ftile0():
                # B-tails for f-pair 0 (row-packed: gate rows 0:64, up
                # rows 64:128), then the SwiGLU consumer into ht
                bt = {0: wsl0(KT), DT: s32[:, KT * 128:KT * 128 + 128]}
                ht_i = ht_pool.tile([128, T], BF16, tag="ht")
                ht_tiles[0] = ht_i
                for m in (0, DT):
                    for c in range(NCH):
                        nc.tensor.matmul(
                            pp0[m][c][:], bt[m][0:R, :],
                            xa_sb[0:R, c * TC:(c + 1) * TC],
                            start=False, stop=True)
                for c in range(NCH):
                    sil = sil_pool.tile([128, TC], F32, tag="sil")
                    nc.scalar.activation(sil[:], pp0[0][c][:], AF.Silu)
                    nc.vector.tensor_mul(ht_i[:, c * TC:(c + 1) * TC],
                                         pp0[DT][c][:], sil[:])

            # ---- f-pair 0 (chains already emitted in the opening)
            dummies(WARM.get("t0", 0))
            close_ftile0()
            dummies(WARM.get("t1", 0))
            for j in range(JT):
                oacc[j] = oacc_pool.tile([128, T], F32, tag="oacc",
                                         name=f"oacc_{rep}_{j}")

            # ---- main mm1 loop over f-pairs (gate m=i, up m=i+32)
            for i in range(1, DT):
                # consumers lagged one pair so PE never waits on fresh DVE
                # output (at i=1 the PE queue must not head-block on ht0,
                # so xa2_mm(0) is emitted after the chains instead)
                if i > 1:
                    emit_xa2_mm(i - 1)
                if i % 4 == 0:
                    mm2_block(i // 4 - 1)

                slabs = {}
                for m in (i, i + DT):
                    if m in pre_slabs:
                        slabs[m] = pre_slabs[m]
                        continue
                    s = wgu_pool.tile([128, SLAB], BF16, tag="wgu")
                    nc.sync.dma_start(out=s[:], in_=wgu[m])
                    slabs[m] = s
                if i == 1:
                    # adbuf is first needed by xa2_mm(0), well after the
                    # opening — keep it off the critical sync stream
                    nc.sync.dma_start(out=adbuf[:], in_=ad[:, :])

                ht_i = ht_pool.tile([128, T], BF16, tag="ht")
                ht_tiles[i] = ht_i
                # both chunks under one weight load per k-tile; B-tail
                # inline per m so the up-chain covers the gate consumers
                pp = {}
                for m in (i, i + DT):
                    s = slabs[m]
                    psl = [ps_a.tile([128, TC], F32, tag="psa",
                                     name=f"ps1_{i}_{m}_{c}")
                           for c in range(NCH)]
                    for k in range(KT):
                        for c in range(NCH):
                            nc.tensor.matmul(
                                psl[c][:], s[:, k * 128:(k + 1) * 128],
                                xsl(k, c), start=(k == 0), stop=False)
                    for c in range(NCH):
                        # lora B-tail folded into the same accumulation
                        nc.tensor.matmul(
                            psl[c][:], s[0:R, KT * 128:KT * 128 + 128],
                            xa_sb[0:R, c * TC:(c + 1) * TC],
                            start=False, stop=True)
                    pp[m] = psl
                for c in range(NCH):
                    sil = sil_pool.tile([128, TC], F32, tag="sil")
                    nc.scalar.activation(sil[:], pp[i][c][:], AF.Silu)
                    nc.vector.tensor_mul(ht_i[:, c * TC:(c + 1) * TC],
                                         pp[i + DT][c][:], sil[:])
                if i == 1:
                    emit_xa2_mm(0)
                if i == DT - 2:
                    # batched load of all 16 bd tiles for the fused block
                    bdall[0] = bd_pool.tile([128, JT * 128], BF16, tag="bd",
                                            name=f"bdall_{rep}")
                    nc.sync.dma_start(out=bdall[0][:], in_=bd[:, :])

            emit_xa2_mm(DT - 1)

            # ---- lora-2: xa2 to SBUF, then final mm2 block fused with
            # the B_d tail and the bf16 output store
            xa2_sb = sm_pool.tile([128, T], BF16, tag="xa2")
            for c in range(NCH):
                nc.vector.tensor_copy(xa2_sb[:, c * TC:(c + 1) * TC], pxa2[c][:])
            mm2_block(NB - 1, fuse_tail=True)

        if loop_n is not None:
            with tc.For_i(0, loop_n):
                emit_rep(0)
        else:
            for rep in range(reps):
                emit_rep(rep)

    nc.compile()
    return nc


def _prep_shared(W_gu, A_gu, B_gu, W_d, A_d, B_d):
    bf = mybir.dt.np(BF16)
    # wgu slab [m, p, SLAB]: cols 0..2048 = W_gu.T tiles, cols 2048..2176
    # rows 0..63 = scaled B_gu.T tile for the same f-tile m
    wgu_t = np.zeros((FT, 128, SLAB), np.float32)
    wgu_t[:, :, :KT * 128] = W_gu.reshape(FT, 128, KT, 128).transpose(
        0, 3, 2, 1).reshape(FT, 128, KT * 128)
    bgu_t = (B_gu * SCALING).reshape(FT, 128, R).transpose(0, 2, 1)
    wgu_t[:, :R, KT * 128:KT * 128 + 128] = bgu_t
    agu_t = np.ascontiguousarray(
        A_gu.T.reshape(KT, 128, R).transpose(1, 0, 2)).reshape(128, KT * R)
    wd_t = np.ascontiguousarray(
        W_d.reshape(JT, 128, NB, 4, 128).transpose(2, 0, 4, 3, 1)
    ).reshape(NB, JT, 128, 4 * 128)
    ad_t = np.ascontiguousarray(
        A_d.T.reshape(DT, 128, R).transpose(1, 0, 2)).reshape(128, DT * R)
    bd_half = (B_d * SCALING).reshape(JT, 128, R).transpose(0, 2, 1)
    bd_t = np.ascontiguousarray(
        np.concatenate([bd_half, bd_half], axis=1)   # [JT, 128, 128]
        .transpose(1, 0, 2)).reshape(128, JT * 128)
    return dict(wgu=wgu_t.astype(bf), agu=agu_t.astype(bf),
                wd=wd_t.astype(bf), ad=ad_t.astype(bf), bd=bd_t.astype(bf))


def kernel(hidden_states, W_gu, A_gu, B_gu, W_d, A_d, B_d):
    bf = mybir.dt.np(BF16)
    hidden_states = np.asarray(hidden_states, dtype=np.float32)
    shared = _prep_shared(*(np.asarray(a, dtype=np.float32)
                            for a in (W_gu, A_gu, B_gu, W_d, A_d, B_d)))

    # per-core xT pre-tiled as [p, k, t] flattened to [128, KT*T]
    xt = np.ascontiguousarray(
        hidden_states.reshape(NCORES, T, KT, 128).transpose(0, 3, 2, 1)
    ).reshape(NCORES, 128, KT * T).astype(bf)

    if "nc" not in _CACHE:
        _CACHE["nc"] = _build()
    nc = _CACHE["nc"]

    in_maps = [dict(shared, xT=xt[c]) for c in range(NCORES)]
    trace = os.environ.get("KERNEL_TRACE", "0") == "1"
    res = run_bass_kernel_spmd(nc, in_maps, list(range(NCORES)), trace=trace)
    _CACHE["last_result"] = res

    out = np.empty((NCORES, T, H), np.float32)
    for c in range(NCORES):
        o = res.results[c]["outT"].astype(np.float32).reshape(JT, 128, T)
        out[c] = o.transpose(2, 0, 1).reshape(T, H)
    return out.reshape(NCORES * T, H)

